# revision 26
# baseline (speedup 1.0000x reference)
"""Criss-cross attention (width=1) Trainium2 Bass kernel — fp8 DoubleRow PV.

Math: for width=1 the criss-cross module is plain softmax attention over all
n keys (the masked diagonal re-enters as the width logit):

    out = gamma * (V @ softmax_j(Q^T K)) + x
    Q = relu(bn1(w_q x)),  K = relu(bn2(w_k x)),  V = relu(bn3(w_v x))

Sharding: 8 cores = (4 batches) x (2 query halves), zero communication.
Each core gets x column-rotated so its 2048 queries are always cols 0:2047
(softmax over keys is order-invariant; K/V use the same key permutation).

Per-core pipeline (ACT-exp is the bound engine):
  x loads as bf16 via gpsimd casting DMAs (the DGE converts f32->bf16 in
  flight; no engine cast pass).  The f32 query half loads separately for
  the residual add.
  Probe (per 128-query chunk): S-layout matmuls against key blocks 0+1,
  DVE row-max -> per-query probe max M_i; PE-transpose into row 32/96 of
  the Q operand.  QK runs with 33 contraction rows ((q, M_i) x (k, -1)) at
  tile_position row offsets {0, 64}, two key blocks per pair into one
  [128,2,512] PSUM tile, yielding s_ij - M_i directly.
  exp: one ACT instruction per pair, bias=-MARGIN, fp8e5 out [128,2,512]
  (256-key probe max + margin keep exp < 57344: no inf).
  PV: fp8 DoubleRow matmuls, contraction 256 (the block pair), lhsT=E e5m2,
  rhs=[V^T | 1] e4m3 -> O^T[q, 0:256] and Z in col 256 at 2x bf16 rate.
  Zero-valued filler DoubleRow matmuls (+= 0) pad PE occupancy through
  ACT-paced gaps so the tensor-engine p-state stays at full clock.
  Epilogue: phase A frees all four O^T PSUM banks (reciprocal(Z)*gamma,
  scale) before phase B (transpose via DMA mid-kernel / PE on the tail,
  residual add on gpsimd, store) so the next block's PV never waits.
"""

import numpy as np
import ml_dtypes

_B, _C, _N, _CR = 4, 256, 4096, 32
_NCORES = 8
_HALF = _N // 2
_EPS = 1e-5
_MARGIN = 4.75
_NJ = _N // 128       # 32 key blocks
_NP = _NJ // 2        # 16 key-block pairs
_NI5 = _HALF // 512   # 4 query super-blocks
_VTW = _C + 1         # 257: V^T columns + ones column for Z
_NWARM = 10
_NFILL = 1            # zero-PV fillers per pair (PE p-state hold)
_DEBUG = False

_BUILD_CACHE: dict = {}


def _build(has_bq: bool, has_bk: bool, has_bv: bool):
    import concourse.mybir as mybir
    import concourse.tile as tile
    from concourse import bacc
    from concourse.masks import make_identity

    f32 = mybir.dt.float32
    bf16 = mybir.dt.bfloat16
    fp8e4 = mybir.dt.float8e4
    fp8e5 = mybir.dt.float8e5
    AF = mybir.ActivationFunctionType
    ALU = mybir.AluOpType
    PM = mybir.MatmulPerfMode
    AX = mybir.AxisListType

    nc = bacc.Bacc("TRN2", target_bir_lowering=False, debug=False)

    x_d = nc.dram_tensor("x", [_C, _N], f32, kind="ExternalInput")
    wq_d = nc.dram_tensor("wq2", [_C, 2 * _CR], bf16, kind="ExternalInput")
    wk_d = nc.dram_tensor("wk4", [_C, 4 * _CR], bf16, kind="ExternalInput")
    wv_d = nc.dram_tensor("wvt", [_C, _C], bf16, kind="ExternalInput")
    g_d = nc.dram_tensor("gvec", [128, 1], f32, kind="ExternalInput")
    bq_d = nc.dram_tensor("bq2", [2 * _CR, 1], f32, kind="ExternalInput") if has_bq else None
    bk_d = nc.dram_tensor("bk4", [4 * _CR, 1], f32, kind="ExternalInput") if has_bk else None
    bv_d = nc.dram_tensor("bv", [1, _C], bf16, kind="ExternalInput") if has_bv else None
    out_d = nc.dram_tensor("out", [_C, _HALF], f32, kind="ExternalOutput")

    with tile.TileContext(nc) as tc:
        with tc.tile_pool(name="persist", bufs=1) as pers, \
             tc.tile_pool(name="work", bufs=2) as work:
            g_sb = pers.tile([128, 1], f32, name="g_sb")
            nc.scalar.dma_start(g_sb, g_d.ap())

            ident = pers.tile([128, 128], bf16, name="ident")
            make_identity(nc, ident)

            wq_sb = pers.tile([128, 4 * _CR], bf16, name="wq_sb")
            nc.scalar.dma_start(wq_sb[:, 0:2 * _CR], wq_d.ap()[0:128, :])
            nc.scalar.dma_start(wq_sb[:, 2 * _CR:4 * _CR], wq_d.ap()[128:256, :])
            wk_sb = pers.tile([128, 8 * _CR], bf16, name="wk_sb")
            nc.scalar.dma_start(wk_sb[:, 0:4 * _CR], wk_d.ap()[0:128, :])
            nc.scalar.dma_start(wk_sb[:, 4 * _CR:8 * _CR], wk_d.ap()[128:256, :])
            wv_sb = pers.tile([128, 2 * _C], bf16, name="wv_sb")
            nc.scalar.dma_start(wv_sb[:, 0:_C], wv_d.ap()[0:128, :])
            nc.scalar.dma_start(wv_sb[:, _C:2 * _C], wv_d.ap()[128:256, :])

            if has_bq:
                bq_sb = pers.tile([2 * _CR, 1], f32, name="bq_sb")
                nc.scalar.dma_start(bq_sb, bq_d.ap())
            if has_bk:
                bk_sb = pers.tile([4 * _CR, 1], f32, name="bk_sb")
                nc.scalar.dma_start(bk_sb, bk_d.ap())
            if has_bv:
                bv_sb = pers.tile([1, _C], bf16, name="bv_sb")
                nc.scalar.dma_start(bv_sb, bv_d.ap())
                ones_row = pers.tile([1, 128], bf16, name="ones_row")
                nc.any.memset(ones_row, 1.0)

            margin_sb = pers.tile([128, 1], f32, name="margin_sb")
            nc.any.memset(margin_sb, -_MARGIN)

            xbf0 = pers.tile([128, _N], bf16, name="xbf0")
            xbf1 = pers.tile([128, _N], bf16, name="xbf1")
            xq0 = pers.tile([128, _HALF], f32, name="xq0")
            xq1 = pers.tile([128, _HALF], f32, name="xq1")
            # pair-packed K operand: rows 0:32 even block, 32 = -1 (probe-max
            # row), 64:96 odd block, 96 = -1; cols = pair*128 + key
            kpk = pers.tile([128, _NP * 128], bf16, name="kpk")
            nc.any.memset(kpk[32:33, :], -1.0)
            nc.any.memset(kpk[96:97, :], -1.0)
            # Q operand: rows 0:32 q, 32 = probe max M_i, replicated at 64:96/96
            qt = pers.tile([128, _HALF], bf16, name="qt")
            # [V^T | 1] in fp8e4, DoubleRow layout [pair, sub-block, col]
            vt = pers.tile([128, _NP, 2, _VTW], fp8e4, name="vt")
            for pp in range(_NP):
                nc.any.memset(vt[:, pp, 0, _C:_C + 1], 1.0)
                nc.any.memset(vt[:, pp, 1, _C:_C + 1], 1.0)
            # zero fp8 operand for p-state filler matmuls
            zfill = pers.tile([128, 2, 128], fp8e5, name="zfill")
            nc.vector.memset(zfill, 0.0)

            def warmup(aps):
                junk = pers.tile([128, 512], bf16, name="junk")
                nc.vector.memset(junk, 0.0)
                warm_ps = aps.tile([128, 2, 512], f32, name="warm_ps", tag="st",
                                   bufs=2)
                for _ in range(_NWARM):
                    nc.tensor.matmul(warm_ps[:, 0, :], ident, junk,
                                     start=True, stop=True)

            def load_chunk(c, pieces=1):
                """bf16 x via gpsimd casting DMAs; f32 query half via sync."""
                w = 512 // pieces
                for i in range(pieces):
                    sl = slice(c * 512 + i * w, c * 512 + (i + 1) * w)
                    nc.gpsimd.dma_start(xbf0[:, sl], x_d.ap()[0:128, sl])
                    nc.gpsimd.dma_start(xbf1[:, sl], x_d.ap()[128:256, sl])
                if c < 4:
                    for i in range(pieces):
                        sl = slice(c * 512 + i * w, c * 512 + (i + 1) * w)
                        nc.sync.dma_start(xq0[:, sl], x_d.ap()[0:128, sl])
                        nc.sync.dma_start(xq1[:, sl], x_d.ap()[128:256, sl])

            def prep_chunk(aps, c):
                """K and V^T for key blocks 4c..4c+3 (pairs 2c, 2c+1).

                Must be EMITTED before qk_exp(2c): tile deps are
                program-order, so a QK read emitted before the k-scatter
                write would read stale SBUF.  The i5=0 loop preps chunk c
                at pair 2c-2 (chunks 0-1 in the preamble)."""
                if c + 2 < _N // 512:
                    load_chunk(c + 2)
                sl = slice(c * 512, (c + 1) * 512)
                kp = aps.tile([128, 512], f32, name="kp", tag="st", bufs=2)
                nc.tensor.matmul(kp, wk_sb[:, 0:4 * _CR], xbf0[:, sl],
                                 start=True, stop=False)
                nc.tensor.matmul(kp, wk_sb[:, 4 * _CR:8 * _CR], xbf1[:, sl],
                                 start=False, stop=True)
                for t in range(4):
                    jb = 4 * c + t
                    pp, hf = divmod(jb, 2)
                    dst = kpk[64 * hf:64 * hf + 32, pp * 128:(pp + 1) * 128]
                    src = kp[32 * t:32 * t + 32, t * 128:(t + 1) * 128]
                    if has_bk:
                        nc.vector.tensor_scalar(
                            dst, src, bk_sb[32 * t:32 * t + 32, :], 0.0,
                            ALU.add, ALU.max)
                    else:
                        nc.vector.tensor_scalar_max(dst, src, 0.0)
                # all four V^T blocks in ONE PSUM alloc (2 banks): less
                # st-tag churn so the i5=0 interleave doesn't stall the PE
                vq = aps.tile([128, 4, _C], f32, name="vq", tag="st", bufs=2)
                for t in range(4):
                    jb = 4 * c + t
                    jsl = slice(jb * 128, (jb + 1) * 128)
                    nc.tensor.matmul(vq[:, t, :], xbf0[:, jsl], wv_sb[:, 0:_C],
                                     start=True, stop=not has_bv)
                    nc.tensor.matmul(vq[:, t, :], xbf1[:, jsl],
                                     wv_sb[:, _C:2 * _C],
                                     start=False, stop=not has_bv)
                    if has_bv:
                        nc.tensor.matmul(vq[:, t, :], ones_row, bv_sb,
                                         start=False, stop=True)
                for t in range(4):
                    jb = 4 * c + t
                    pp, hf = divmod(jb, 2)
                    dst = vt[:, pp, hf, 0:_C]
                    # all relu+fp8 casts on DVE: ACT stays exp-only so the
                    # bound engine never waits behind prep work
                    nc.vector.tensor_scalar_max(dst, vq[:, t, :], 0.0)

            def qp_i5(aps, i5):
                isl = slice(i5 * 512, (i5 + 1) * 512)
                qp = aps.tile([64, 512], f32, name="qp", tag="st", bufs=2)
                nc.tensor.matmul(qp, wq_sb[:, 0:2 * _CR], xbf0[:, isl],
                                 start=True, stop=False)
                nc.tensor.matmul(qp, wq_sb[:, 2 * _CR:4 * _CR], xbf1[:, isl],
                                 start=False, stop=True)
                for r in range(2):
                    dst = qt[64 * r:64 * r + 32, isl]
                    src = qp[32 * r:32 * r + 32, :]
                    if has_bq:
                        nc.vector.tensor_scalar(
                            dst, src, bq_sb[32 * r:32 * r + 32, :], 0.0,
                            ALU.add, ALU.max)
                    else:
                        nc.vector.tensor_scalar_max(dst, src, 0.0)

            def probe(aps, i5, c):
                """Per-query probe max over key blocks 0+1 -> qt rows 32/96.

                256 probe keys bound the worst per-query gap (max + margin
                must stay under e5m2's inf threshold, exp arg <= 11.03).
                Two sub-tiles = two PSUM banks: concurrent groups at PE row
                positions 0 and 64 must not share a bank."""
                col = i5 * 512 + c * 128
                pr = aps.tile([128, 2, 512], f32, name="pr", tag="st", bufs=2)
                nc.tensor.matmul(pr[:, 0, 0:128], qt[0:32, col:col + 128],
                                 kpk[0:32, 0:128], start=True, stop=True,
                                 tile_position=(0, 0))
                nc.tensor.matmul(pr[:, 1, 0:128], qt[64:96, col:col + 128],
                                 kpk[64:96, 0:128], start=True, stop=True,
                                 tile_position=(64, 0))
                m = work.tile([128, 1], bf16, name="m", tag="mpr", bufs=4)
                nc.vector.tensor_reduce(m, pr[:, 0:2, 0:128], AX.XY, ALU.max)
                trow = aps.tile([1, 128], bf16, name="trow", tag="st", bufs=2)
                nc.tensor.transpose(trow, m, ident)
                nc.vector.tensor_copy(qt[32:33, col:col + 128], trow)
                nc.vector.tensor_copy(qt[96:97, col:col + 128], trow)

            def probe_batch(aps, i5):
                """All four probe chunks of an i5 in one PSUM alloc."""
                base = i5 * 512
                pr = aps.tile([128, 2, 512], f32, name="prb", tag="st", bufs=2)
                for c in range(4):
                    csl = slice(c * 128, (c + 1) * 128)
                    col = base + c * 128
                    nc.tensor.matmul(pr[:, 0, csl], qt[0:32, col:col + 128],
                                     kpk[0:32, 0:128], start=True, stop=True,
                                     tile_position=(0, 0))
                    nc.tensor.matmul(pr[:, 1, csl], qt[64:96, col:col + 128],
                                     kpk[64:96, 0:128], start=True, stop=True,
                                     tile_position=(64, 0))
                ms = []
                for c in range(4):
                    csl = slice(c * 128, (c + 1) * 128)
                    m = work.tile([128, 1], bf16, name="m", tag="mpr", bufs=4)
                    nc.vector.tensor_reduce(m, pr[:, 0:2, csl], AX.XY, ALU.max)
                    ms.append(m)
                for c, m in enumerate(ms):
                    col = base + c * 128
                    trow = aps.tile([1, 128], bf16, name="trow", tag="st",
                                    bufs=2)
                    nc.tensor.transpose(trow, m, ident)
                    nc.vector.tensor_copy(qt[32:33, col:col + 128], trow)
                    nc.vector.tensor_copy(qt[96:97, col:col + 128], trow)

            with tc.tile_pool(name="att_ps", space="PSUM", bufs=1) as aps:
                load_chunk(0, pieces=4)
                load_chunk(1, pieces=2)
                warmup(aps)
                qp_i5(aps, 0)
                prep_chunk(aps, 0)
                probe_batch(aps, 0)
                prep_chunk(aps, 1)

                for i5 in range(_NI5):
                    isl = slice(i5 * 512, (i5 + 1) * 512)
                    last_i5 = i5 == _NI5 - 1
                    ots = [
                        aps.tile([128, _VTW], f32, name=f"ot{s}", tag=f"ot{s}",
                                 bufs=1)
                        for s in range(4)
                    ]
                    e_tiles = [None] * _NP
                    onrms = [None] * 4

                    def qk_exp(p):
                        st = aps.tile([128, 2, 512], f32, name="st", tag="st",
                                      bufs=2)
                        nc.tensor.matmul(
                            st[:, 0, :], kpk[0:33, p * 128:(p + 1) * 128],
                            qt[0:33, isl], start=True, stop=True,
                            tile_position=(0, 0))
                        nc.tensor.matmul(
                            st[:, 1, :], kpk[64:97, p * 128:(p + 1) * 128],
                            qt[64:97, isl], start=True, stop=True,
                            tile_position=(64, 0))
                        e = work.tile([128, 2, 512], fp8e5, name="e_sb",
                                      tag="e", bufs=4)
                        nc.scalar.activation(e, st, AF.Exp, bias=margin_sb)
                        e_tiles[p] = e

                    def epi_a(s):
                        """Free ot bank s: reciprocal+scale only (all DVE)."""
                        rz = work.tile([128, 1], f32, name="rz", tag=f"rz{s}",
                                       bufs=2)
                        nc.vector.reciprocal(rz, ots[s][:, _C:_C + 1])
                        rzg = work.tile([128, 1], f32, name="rzg",
                                        tag=f"rzg{s}", bufs=2)
                        nc.vector.tensor_scalar_mul(rzg, rz, g_sb)
                        onrm = work.tile([128, _C], bf16, name="onrm",
                                         tag=f"onrm{s}", bufs=2)
                        nc.vector.tensor_scalar_mul(onrm, ots[s][:, 0:_C], rzg)
                        onrms[s] = onrm

                    def epi_b(s):
                        onrm = onrms[s]
                        i0 = i5 * 512 + s * 128
                        for chh in range(2):
                            xq_t = xq0 if chh == 0 else xq1
                            res = work.tile([128, 128], f32, name="res",
                                            tag="res", bufs=8)
                            if last_i5:
                                tp = aps.tile([128, 128], bf16, name="tp",
                                              tag="st", bufs=2)
                                nc.tensor.transpose(
                                    tp, onrm[:, chh * 128:(chh + 1) * 128],
                                    ident)
                                nc.vector.tensor_add(res, tp,
                                                     xq_t[:, i0:i0 + 128])
                                nc.sync.dma_start(
                                    out_d.ap()[chh * 128:(chh + 1) * 128,
                                               i0:i0 + 128], res)
                            else:
                                tT = work.tile([128, 128], bf16, name="tT",
                                               tag="tT", bufs=8)
                                nc.sync.dma_start(
                                    tT, onrm[:, chh * 128:(chh + 1) * 128],
                                    transpose=True)
                                # add on gpsimd: keeps the in-order DVE queue
                                # free of transpose-gated work (ot frees and
                                # probe maxes must not wait behind it)
                                nc.gpsimd.tensor_add(res, tT,
                                                     xq_t[:, i0:i0 + 128])
                                nc.gpsimd.dma_start(
                                    out_d.ap()[chh * 128:(chh + 1) * 128,
                                               i0:i0 + 128], res)

                    def pv(p, tail=False):
                        e = e_tiles[p]
                        for s in range(4):
                            nc.tensor.matmul(
                                ots[s], e[:, :, s * 128:(s + 1) * 128],
                                vt[:, p, :, :], start=(p == 0),
                                stop=(p == _NP - 1), perf_mode=PM.DoubleRow)
                            if tail:
                                epi_a(s)
                        if not tail and i5 > 0:
                            for f in range(_NFILL):
                                nc.tensor.matmul(
                                    ots[(p + f) % 4], zfill, vt[:, p, :, :],
                                    start=False, stop=False,
                                    perf_mode=PM.DoubleRow)
                        e_tiles[p] = None

                    for p in range(_NP):
                        qk_exp(p)
                        if i5 == 0 and p % 2 == 0 and 2 <= p <= 12:
                            prep_chunk(aps, p // 2 + 1)
                        if not last_i5:
                            if p == 8:
                                qp_i5(aps, i5 + 1)
                            elif 10 <= p <= 13:
                                probe(aps, i5 + 1, p - 10)
                        if p > 0:
                            pv(p - 1)
                    pv(_NP - 1, tail=True)
                    for s in range(4):
                        epi_b(s)

    nc.compile()
    return nc


def _get_nc(has_bq, has_bk, has_bv):
    key = (has_bq, has_bk, has_bv)
    if key not in _BUILD_CACHE:
        _BUILD_CACHE[key] = _build(*key)
    return _BUILD_CACHE[key]


def kernel(x, w_q, w_k, w_v,
           bn1_scale, bn1_bias, bn1_mean, bn1_var,
           bn2_scale, bn2_bias, bn2_mean, bn2_var,
           bn3_scale, bn3_bias, bn3_mean, bn3_var,
           gamma, _trace=False):
    from concourse.bass_utils import run_bass_kernel_spmd

    x = np.asarray(x, dtype=np.float32)
    gamma_f = float(np.asarray(gamma).reshape(-1)[0])
    bf = ml_dtypes.bfloat16

    def fold(w, s, b, m, v):
        a = np.asarray(s, np.float32) / np.sqrt(np.asarray(v, np.float32) + _EPS)
        return (np.asarray(w, np.float32) * a[:, None],
                np.asarray(b, np.float32) - np.asarray(m, np.float32) * a)

    wqf, bq = fold(w_q, bn1_scale, bn1_bias, bn1_mean, bn1_var)
    wkf, bk = fold(w_k, bn2_scale, bn2_bias, bn2_mean, bn2_var)
    wvf, bv = fold(w_v, bn3_scale, bn3_bias, bn3_mean, bn3_var)
    has_bq = bool(np.any(bq != 0.0))
    has_bk = bool(np.any(bk != 0.0))
    has_bv = bool(np.any(bv != 0.0))

    nc = _get_nc(has_bq, has_bk, has_bv)

    wq2 = np.tile(np.ascontiguousarray(wqf.T), (1, 2)).astype(bf)
    wk4 = np.tile(np.ascontiguousarray(wkf.T), (1, 4)).astype(bf)
    wvt = np.ascontiguousarray(wvf.T).astype(bf)
    gvec = np.full((128, 1), gamma_f, dtype=np.float32)

    in_maps = []
    for core in range(_NCORES):
        b, h = divmod(core, 2)
        if h == 0:
            xc = np.ascontiguousarray(x[b])
        else:
            # rotate columns so this core's queries are always cols 0:2047;
            # key order is permuted identically for K and V so softmax-over-
            # keys is unchanged.
            xc = np.ascontiguousarray(
                np.concatenate([x[b][:, _HALF:], x[b][:, :_HALF]], axis=1))
        m = {"x": xc, "wq2": wq2, "wk4": wk4, "wvt": wvt, "gvec": gvec}
        if has_bq:
            m["bq2"] = np.ascontiguousarray(np.tile(bq, 2).reshape(2 * _CR, 1))
        if has_bk:
            m["bk4"] = np.ascontiguousarray(np.tile(bk, 4).reshape(4 * _CR, 1))
        if has_bv:
            m["bv"] = np.ascontiguousarray(bv.reshape(1, _C)).astype(bf)
        in_maps.append(m)

    res = run_bass_kernel_spmd(nc, in_maps, core_ids=list(range(_NCORES)),
                               trace=_trace)

    out = np.empty((_B, _C, _N), dtype=np.float32)
    for core in range(_NCORES):
        b, h = divmod(core, 2)
        out[b, :, h * _HALF:(h + 1) * _HALF] = res.results[core]["out"]
    if _trace:
        kernel.last_results = res
    return out
